# revision 1
# baseline (speedup 1.0000x reference)
"""BitSPPF kernel for Trainium2 (8 NeuronCores, data-parallel over batch).

Pipeline per core (4 images):
  cv1 (1x1 ternary-quantized conv) -> BN+SiLU (fused in ACT engine)
  -> 3x chained 5x5 maxpool (separable max trees on DVE, bf16)
  -> concat -> cv2 (1x1 ternary conv) -> BN+SiLU -> DRAM.

Ternary weights {-1,0,+1} are exact in bf16; the BitNet scale s and BN
affine fold into per-output-channel (scale, bias) applied by the ACT
engine's Silu(scale*x + bias).
"""

import os
import sys

for _p in ("/opt/trn_rl_repo",):
    if _p not in sys.path and os.path.isdir(_p):
        sys.path.insert(0, _p)

import numpy as np
import ml_dtypes

import concourse.bass as bass
import concourse.tile as tile
from concourse import bacc, mybir

BF16 = mybir.dt.bfloat16
F32 = mybir.dt.float32
NPBF16 = ml_dtypes.bfloat16

# Problem shapes (hardcoded per spec)
B, C1, H, W = 32, 1024, 40, 40
HID, C2 = 512, 1024
S = H * W  # 1600
N_CORES = 8
BL = B // N_CORES  # images per core

NEG = -3.0e38  # effectively -inf for maxpool padding, finite in bf16

EPS = 1e-8
BN_EPS = 1e-5


def _pools_chain(nc, P, HX, M2, Pout, padded_out):
    """One 5x5 stride-1 pad-2 maxpool: P -> Pout.

    P: [128, 40, 44] bf16, data in cols 2..41, cols {0,1,42,43} = NEG.
    HX: [128, 44, 40] scratch; rows {0,1,42,43} pre-set to NEG.
    M2: [128, 44, 44] scratch.
    Pout: [128, 40, 44] (padded_out=True, data to cols 2..41)
          or [128, 40, 40] (padded_out=False).
    """
    # x-direction 5-window into HX rows 2..41:
    #   m2[y, c] = max(P[y, c], P[y, c+1])            c in 0..42
    #   HX[2+y, x] = max(m2[y,x], m2[y,x+2], P[y,x+4])
    nc.vector.tensor_max(M2[:, 0:40, 0:43], P[:, :, 0:43], P[:, :, 1:44])
    nc.vector.tensor_max(HX[:, 2:42, :], M2[:, 0:40, 0:40], M2[:, 0:40, 2:42])
    nc.vector.tensor_max(HX[:, 2:42, :], HX[:, 2:42, :], P[:, :, 4:44])
    # y-direction 5-window:
    #   m2y[j, x] = max(HX[j, x], HX[j+1, x])         j in 0..42
    #   out[y, x] = max(m2y[y], m2y[y+2], HX[y+4])
    nc.vector.tensor_max(M2[:, 0:43, 0:40], HX[:, 0:43, :], HX[:, 1:44, :])
    if padded_out:
        ov = Pout[:, :, 2:42]
    else:
        ov = Pout[:, :, :]
    nc.vector.tensor_max(ov, M2[:, 0:40, 0:40], M2[:, 2:42, 0:40])
    nc.vector.tensor_max(ov, ov, HX[:, 4:44, :])


def _build_nc(bl=BL):
    nc = bacc.Bacc(trn_type="TRN2", debug=False)

    xq_d = nc.dram_tensor("xq", [bl, C1, S], BF16, kind="ExternalInput")
    w1t_d = nc.dram_tensor("w1t", [C1, HID], BF16, kind="ExternalInput")
    w2t_d = nc.dram_tensor("w2t", [4 * HID, C2], BF16, kind="ExternalInput")
    sc1_d = nc.dram_tensor("sc1", [HID], F32, kind="ExternalInput")
    bi1_d = nc.dram_tensor("bi1", [HID], F32, kind="ExternalInput")
    sc2_d = nc.dram_tensor("sc2", [C2], F32, kind="ExternalInput")
    bi2_d = nc.dram_tensor("bi2", [C2], F32, kind="ExternalInput")
    out_d = nc.dram_tensor("out", [bl, C2, S], F32, kind="ExternalOutput")

    KT1 = C1 // 128       # 8 k-tiles for cv1
    MT1 = HID // 128      # 4 m-tiles (= pool channel tiles)
    KT2 = 4 * HID // 128  # 16 k-tiles for cv2
    MT2 = C2 // 128       # 8 m-tiles for cv2
    NQ = 4                # spatial quarters of 400 cols (10 rows of 40)
    QW = S // NQ          # 400

    xv = xq_d.ap().rearrange("b (kt p) s -> b p kt s", p=128)
    ov = out_d.ap().rearrange("b (mt p) s -> b p mt s", p=128)

    # CoreSim doesn't implement Silu; allow substituting Sigmoid for
    # wiring-validation sim runs (numerics then differ by design).
    if os.environ.get("BITSPPF_SIM_ACT") == "sigmoid":
        silu = mybir.ActivationFunctionType.Sigmoid
    else:
        silu = mybir.ActivationFunctionType.Silu

    with tile.TileContext(nc) as tc:
        with (
            tc.tile_pool(name="const", bufs=1) as const,
            tc.tile_pool(name="xin", bufs=3) as xin,
            tc.tile_pool(name="pbuf0", bufs=4 * MT1) as pbuf0,
            tc.tile_pool(name="pbuf", bufs=2 * MT1) as pbuf,
            tc.tile_pool(name="work", bufs=1) as work,
            tc.tile_pool(name="osb", bufs=2) as osb,
            tc.tile_pool(name="ps1", bufs=2, space="PSUM") as ps1p,
            tc.tile_pool(name="ps2", bufs=3, space="PSUM") as ps2p,
        ):
            # Pre-warm the ACT engine's Silu spline tables (~2.7us load)
            # during the initial DMA window instead of at the first real
            # activation.
            warm = const.tile([128, 2], F32)
            nc.vector.memset(warm, 0.0)
            nc.scalar.activation(out=warm, in_=warm, func=silu)

            # Load only what cv1(0) needs before its matmuls; the 4MB w2
            # load would otherwise delay the first matmul by ~tens of us.
            w1_sb = const.tile([128, KT1, HID], BF16)
            nc.sync.dma_start(w1_sb, w1t_d.ap().rearrange("(kt p) m -> p kt m", p=128))
            sc1_sb = const.tile([128, MT1], F32)
            nc.sync.dma_start(sc1_sb, sc1_d.ap().rearrange("(t p) -> p t", p=128))
            bi1_sb = const.tile([128, MT1], F32)
            nc.sync.dma_start(bi1_sb, bi1_d.ap().rearrange("(t p) -> p t", p=128))

            def load_cv2_consts():
                w2_sb = const.tile([128, KT2, C2], BF16)
                nc.sync.dma_start(
                    w2_sb, w2t_d.ap().rearrange("(kt p) m -> p kt m", p=128)
                )
                sc2_sb = const.tile([128, MT2], F32)
                nc.sync.dma_start(sc2_sb, sc2_d.ap().rearrange("(t p) -> p t", p=128))
                bi2_sb = const.tile([128, MT2], F32)
                nc.sync.dma_start(bi2_sb, bi2_d.ap().rearrange("(t p) -> p t", p=128))
                return w2_sb, sc2_sb, bi2_sb

            # PE HAM warm-up: keep the PE activity window busy from the
            # moment the (tiny, early-landing) sc1 constants arrive until the
            # first real matmul, so the clock gate is already at 8/8 when it
            # issues (the cold ramp shows as 333ns gaps in the trace). Phase 1
            # runs ~4us of tiny fp32 matmuls on sc1; phase 2 bridges the
            # remaining wait on w1 itself.
            wps = ps1p.tile([128, 512], F32, tag="ps1")
            for _i in range(80):
                nc.tensor.matmul(
                    wps[0:4, 0:4],
                    sc1_sb,
                    sc1_sb,
                    start=True,
                    stop=True,
                )
            for _i in range(30):
                nc.tensor.matmul(
                    wps[:, 0:32],
                    w1_sb[:, 0, 0:128],
                    w1_sb[:, 0, 0:32],
                    start=True,
                    stop=True,
                )

            pimg = {}  # b -> [P0 list, P1 list, P2 list, P3 list]

            def emit_cv1(b):
                """cv1 + fused BN/SiLU; writes h into padded P0 buffers."""
                P0 = []
                for ct in range(MT1):
                    p0 = pbuf0.tile([128, 40, 44], BF16, tag="P0")
                    nc.gpsimd.memset(p0[:, :, 0:2], NEG)
                    nc.gpsimd.memset(p0[:, :, 42:44], NEG)
                    P0.append(p0)
                pimg[b] = [P0, None, None, None]
                for q in range(NQ):
                    xs = xin.tile([128, KT1, QW], BF16, tag="x")
                    nc.sync.dma_start(xs, xv[b][:, :, q * QW:(q + 1) * QW])
                    for mt in range(MT1):
                        ps = ps1p.tile([128, 512], F32, tag="ps1")
                        for kt in range(KT1):
                            nc.tensor.matmul(
                                ps[:, :QW],
                                w1_sb[:, kt, mt * 128:(mt + 1) * 128],
                                xs[:, kt, :],
                                start=(kt == 0),
                                stop=(kt == KT1 - 1),
                            )
                        nc.scalar.activation(
                            out=P0[mt][:, q * 10:(q + 1) * 10, 2:42],
                            in_=ps[:, :QW],
                            func=silu,
                            bias=bi1_sb[:, mt:mt + 1],
                            scale=sc1_sb[:, mt:mt + 1],
                        )

            def emit_pools(b):
                P0 = pimg[b][0]
                P1, P2, P3 = [], [], []
                for ct in range(MT1):
                    HX = work.tile([128, 44, 40], BF16, tag="HX")
                    M2 = work.tile([128, 44, 44], BF16, tag="M2")
                    nc.gpsimd.memset(HX[:, 0:2, :], NEG)
                    nc.gpsimd.memset(HX[:, 42:44, :], NEG)
                    p1 = pbuf.tile([128, 40, 44], BF16, tag="P1")
                    p2 = pbuf.tile([128, 40, 44], BF16, tag="P2")
                    p3 = pbuf.tile([128, 40, 40], BF16, tag="P3")
                    for pp in (p1, p2):
                        nc.gpsimd.memset(pp[:, :, 0:2], NEG)
                        nc.gpsimd.memset(pp[:, :, 42:44], NEG)
                    _pools_chain(nc, P0[ct], HX, M2, p1, True)
                    _pools_chain(nc, p1, HX, M2, p2, True)
                    _pools_chain(nc, p2, HX, M2, p3, False)
                    P1.append(p1)
                    P2.append(p2)
                    P3.append(p3)
                pimg[b][1] = P1
                pimg[b][2] = P2
                pimg[b][3] = P3

            def rhs_view(b, kt, nt):
                level, ct = kt // MT1, kt % MT1
                buf = pimg[b][level][ct]
                if level < 3:
                    return buf[:, nt * 10:(nt + 1) * 10, 2:42]
                return buf[:, nt * 10:(nt + 1) * 10, :]

            def emit_cv2(b):
                for mt2 in range(MT2):
                    psA = ps2p.tile([128, 2, 512], F32, tag="ps2")
                    psB = ps2p.tile([128, 2, 512], F32, tag="ps2")
                    for kt in range(KT2):
                        lhs = w2_sb[:, kt, mt2 * 128:(mt2 + 1) * 128]
                        st = kt == 0
                        sp = kt == KT2 - 1
                        nc.tensor.matmul(psA[:, 0, :QW], lhs, rhs_view(b, kt, 0),
                                         start=st, stop=sp)
                        nc.tensor.matmul(psA[:, 1, :QW], lhs, rhs_view(b, kt, 1),
                                         start=st, stop=sp)
                        nc.tensor.matmul(psB[:, 0, :QW], lhs, rhs_view(b, kt, 2),
                                         start=st, stop=sp)
                        nc.tensor.matmul(psB[:, 1, :QW], lhs, rhs_view(b, kt, 3),
                                         start=st, stop=sp)
                    oa = osb.tile([128, 800], F32, tag="o")
                    nc.scalar.activation(
                        out=oa, in_=psA[:, :, :QW], func=silu,
                        bias=bi2_sb[:, mt2:mt2 + 1], scale=sc2_sb[:, mt2:mt2 + 1],
                    )
                    nc.sync.dma_start(ov[b][:, mt2, 0:800], oa)
                    ob = osb.tile([128, 800], F32, tag="o")
                    nc.scalar.activation(
                        out=ob, in_=psB[:, :, :QW], func=silu,
                        bias=bi2_sb[:, mt2:mt2 + 1], scale=sc2_sb[:, mt2:mt2 + 1],
                    )
                    nc.sync.dma_start(ov[b][:, mt2, 800:1600], ob)

            # Software pipeline: cv2(b) is emitted two images behind cv1(b)
            # so the PE always has cv1 work while an image's pool chain
            # completes on DVE/GPSIMD (needs 3 images of live P0 slots).
            lag = 3 if bl > 3 else (2 if bl > 2 else 1)
            w2_refs = None
            for b in range(bl):
                emit_cv1(b)
                if b == 0:
                    w2_refs = load_cv2_consts()
                    w2_sb, sc2_sb, bi2_sb = w2_refs
                if b >= lag:
                    emit_cv2(b - lag)
                emit_pools(b)
            for b in range(max(0, bl - lag), bl):
                emit_cv2(b)

    nc.compile()
    return nc


_NC_CACHE = {}


def _get_nc(bl=BL):
    if bl not in _NC_CACHE:
        _NC_CACHE[bl] = _build_nc(bl)
    return _NC_CACHE[bl]


def _prep(inputs):
    """Host-side: quantize weights to ternary, fold BitNet scale + BN into
    per-channel (scale, bias), cast activations/weights to bf16."""
    x = np.asarray(inputs["x"], dtype=np.float32)
    w1 = np.asarray(inputs["w1"], dtype=np.float32)
    w2 = np.asarray(inputs["w2"], dtype=np.float32)
    g1 = np.asarray(inputs["g1"], dtype=np.float32)
    b1 = np.asarray(inputs["b1"], dtype=np.float32)
    m1 = np.asarray(inputs["m1"], dtype=np.float32)
    v1 = np.asarray(inputs["v1"], dtype=np.float32)
    g2 = np.asarray(inputs["g2"], dtype=np.float32)
    b2 = np.asarray(inputs["b2"], dtype=np.float32)
    m2 = np.asarray(inputs["m2"], dtype=np.float32)
    v2 = np.asarray(inputs["v2"], dtype=np.float32)

    def fold(w, g, b, m, v):
        s = np.float32(max(np.median(np.abs(w)), EPS))
        t = np.clip(np.round(w / s), -1.0, 1.0).astype(np.float32)
        inv = g / np.sqrt(v + BN_EPS)
        scale = (s * inv).astype(np.float32)
        bias = (b - m * inv).astype(np.float32)
        return np.ascontiguousarray(t.T).astype(NPBF16), scale, bias

    w1t, sc1, bi1 = fold(w1, g1, b1, m1, v1)
    w2t, sc2, bi2 = fold(w2, g2, b2, m2, v2)

    xq = x.reshape(B, C1, S).astype(NPBF16)
    shared = dict(w1t=w1t, w2t=w2t, sc1=sc1, bi1=bi1, sc2=sc2, bi2=bi2)
    in_maps = []
    for d in range(N_CORES):
        m = dict(shared)
        m["xq"] = np.ascontiguousarray(xq[d * BL:(d + 1) * BL])
        in_maps.append(m)
    return in_maps


def _install_ntff_hook():
    """The agent image's antenv lacks axon_hooks; synthesize it so
    run_bass_kernel_spmd(trace=True) can capture NTFF profiles via the
    axon .so's C ABI (same mechanism trn_boot would install)."""
    import types

    try:
        import antenv.axon_hooks  # noqa: F401

        return
    except ImportError:
        pass
    try:
        import antenv

        bootdir = "/root/.axon_site/trn_agent_boot"
        if bootdir not in sys.path and os.path.isdir(bootdir):
            sys.path.insert(0, bootdir)
        import trn_boot

        hook = trn_boot._ntff_profile_via_ctypes("/opt/axon/libaxon_pjrt.so")
        mod = types.ModuleType("antenv.axon_hooks")
        state = {"h": hook}
        mod.get_axon_ntff_profile_hook = lambda: state["h"]
        mod.set_axon_ntff_profile_hook = lambda h: state.update(h=h)
        sys.modules["antenv.axon_hooks"] = mod
        antenv.axon_hooks = mod
    except Exception as e:  # profiling is best-effort; execution still works
        print(f"ntff hook install failed: {e}", file=sys.stderr)


def _run(inputs, trace=False):
    from concourse import bass_utils

    if trace:
        _install_ntff_hook()
    nc = _get_nc()
    in_maps = _prep(inputs)
    import time

    res = None
    for attempt, delay in ((0, 5), (1, 20), (2, 0)):
        try:
            res = bass_utils.run_bass_kernel_spmd(
                nc, in_maps, core_ids=list(range(N_CORES)), trace=trace,
            )
            break
        except Exception as e:  # transient device errors happen; back off
            if attempt == 2:
                raise
            print(
                f"run_bass_kernel_spmd failed ({type(e).__name__}); "
                f"retrying in {delay}s",
                file=sys.stderr,
            )
            time.sleep(delay)
    assert res is not None
    outs = [res.results[d]["out"] for d in range(N_CORES)]
    full = np.concatenate(outs, axis=0).reshape(B, C2, H, W).astype(np.float32)
    return full, res


def kernel(**inputs):
    full, _ = _run(inputs, trace=False)
    return full


def run_traced(**inputs):
    full, res = _run(inputs, trace=True)
    return full, res.exec_time_ns



# revision 5
# speedup vs baseline: 1.0287x; 1.0287x over previous
"""BitSPPF kernel for Trainium2 (8 NeuronCores, data-parallel over batch).

Pipeline per core (4 images):
  cv1 (1x1 ternary-quantized conv, bf16) -> BN+SiLU (ACT engine)
  -> 3x chained 5x5 maxpool (separable max trees on DVE, bf16)
  -> fp8 re-encode of the SPPF concat -> cv2 in fp8 DoubleRow (2x PE
  throughput) -> BN+SiLU -> DRAM.

cv2's fp8 precision is recovered by two tricks, both free at runtime:
 1. Delta blocks: cat [h,y1,y2,y3] @ [Wa|Wb|Wc|Wd]^T is rewritten as
    [h, y1, y2-y1, y3-y2] @ [Wa|Wb+Wc+Wd|Wc+Wd|Wd]^T (exact identity for
    ternary weights; summed weights stay integers in [-3,3], exact in fp8).
    The delta blocks have small magnitude -> small absolute fp8 error.
 2. Per-channel mean centering: each block is stored as (v - c) in fp8
    with c folded back via the host-computed bias correction
    W'@c added to cv2's BN bias. The subtraction itself rides for free
    on the ACT engine's Identity(scale*x + bias) conversion op.
Calibration constants c come from a 2-image host-side pass in _prep.
"""

import os
import sys

for _p in ("/opt/trn_rl_repo",):
    if _p not in sys.path and os.path.isdir(_p):
        sys.path.insert(0, _p)

import numpy as np
import ml_dtypes

import concourse.bass as bass
import concourse.tile as tile
from concourse import bacc, mybir

BF16 = mybir.dt.bfloat16
F32 = mybir.dt.float32
FP8 = mybir.dt.float8e4
NPBF16 = ml_dtypes.bfloat16
NPFP8 = ml_dtypes.float8_e4m3

# Problem shapes (hardcoded per spec)
B, C1, H, W = 32, 1024, 40, 40
HID, C2 = 512, 1024
S = H * W  # 1600
N_CORES = 8
BL = B // N_CORES  # images per core

NEG = -3.0e38  # effectively -inf for maxpool padding, finite in bf16

EPS = 1e-8
BN_EPS = 1e-5

DR = mybir.MatmulPerfMode.DoubleRow


def _pools_chain(nc, P, HX, M2, Pout, padded_out):
    """One 5x5 stride-1 pad-2 maxpool: P -> Pout.

    P: [128, 40, 44] bf16, data in cols 2..41, cols {0,1,42,43} = NEG.
    HX: [128, 44, 40] scratch; rows {0,1,42,43} pre-set to NEG.
    M2: [128, 44, 44] scratch.
    Pout: [128, 40, 44] (padded_out=True, data to cols 2..41)
          or [128, 40, 40] (padded_out=False).
    """
    nc.vector.tensor_max(M2[:, 0:40, 0:43], P[:, :, 0:43], P[:, :, 1:44])
    nc.vector.tensor_max(HX[:, 2:42, :], M2[:, 0:40, 0:40], M2[:, 0:40, 2:42])
    nc.vector.tensor_max(HX[:, 2:42, :], HX[:, 2:42, :], P[:, :, 4:44])
    nc.vector.tensor_max(M2[:, 0:43, 0:40], HX[:, 0:43, :], HX[:, 1:44, :])
    if padded_out:
        ov = Pout[:, :, 2:42]
    else:
        ov = Pout[:, :, :]
    nc.vector.tensor_max(ov, M2[:, 0:40, 0:40], M2[:, 2:42, 0:40])
    nc.vector.tensor_max(ov, ov, HX[:, 4:44, :])


def _build_nc(bl=BL):
    nc = bacc.Bacc(trn_type="TRN2", debug=False)

    xq_d = nc.dram_tensor("xq", [bl, C1, S], BF16, kind="ExternalInput")
    w1t_d = nc.dram_tensor("w1t", [C1, HID], BF16, kind="ExternalInput")
    w2t_d = nc.dram_tensor("w2t", [4 * HID, C2], FP8, kind="ExternalInput")
    sc1_d = nc.dram_tensor("sc1", [HID], F32, kind="ExternalInput")
    bi1_d = nc.dram_tensor("bi1", [HID], F32, kind="ExternalInput")
    sc2_d = nc.dram_tensor("sc2", [C2], F32, kind="ExternalInput")
    bi2_d = nc.dram_tensor("bi2", [C2], F32, kind="ExternalInput")
    cng_d = nc.dram_tensor("cng", [16 * 128], F32, kind="ExternalInput")
    out_d = nc.dram_tensor("out", [bl, C2, S], F32, kind="ExternalOutput")

    KT1 = C1 // 128       # 8 k-tiles for cv1
    MT1 = HID // 128      # 4 m-tiles (= pool channel tiles)
    KT2 = 4 * HID // 128  # 16 k-subtiles for cv2
    KP2 = KT2 // 2        # 8 fp8 DoubleRow pairs
    MT2 = C2 // 128       # 8 m-tiles for cv2
    NQ = 4                # spatial quarters of 400 cols (10 rows of 40)
    QW = S // NQ          # 400

    xv = xq_d.ap().rearrange("b (kt p) s -> b p kt s", p=128)
    ov = out_d.ap().rearrange("b (mt p) s -> b p mt s", p=128)

    # CoreSim doesn't implement Silu; allow substituting Sigmoid for
    # wiring-validation sim runs (numerics then differ by design).
    if os.environ.get("BITSPPF_SIM_ACT") == "sigmoid":
        silu = mybir.ActivationFunctionType.Sigmoid
    else:
        silu = mybir.ActivationFunctionType.Silu
    ident = mybir.ActivationFunctionType.Identity

    with tile.TileContext(nc) as tc:
        with (
            tc.tile_pool(name="const", bufs=1) as const,
            tc.tile_pool(name="xin", bufs=2) as xin,
            tc.tile_pool(name="pbuf0", bufs=2 * MT1 - 1) as pbuf0,
            tc.tile_pool(name="pbuf", bufs=6) as pbuf,
            tc.tile_pool(name="v8p", bufs=3) as v8p,
            tc.tile_pool(name="work", bufs=1) as work,
            tc.tile_pool(name="osb", bufs=2) as osb,
            tc.tile_pool(name="ps1", bufs=2, space="PSUM") as ps1p,
            tc.tile_pool(name="ps2", bufs=3, space="PSUM") as ps2p,
        ):
            # Pre-warm the ACT engine's Silu spline tables (~2.7us load)
            # during the initial DMA window instead of at the first real
            # activation.
            warm = const.tile([128, 2], F32)
            nc.vector.memset(warm, 0.0)
            nc.scalar.activation(out=warm, in_=warm, func=silu)

            # Load only what cv1(0) needs before its matmuls; the w2
            # load would otherwise delay the first matmul.
            w1_sb = const.tile([128, KT1, HID], BF16)
            nc.sync.dma_start(w1_sb, w1t_d.ap().rearrange("(kt p) m -> p kt m", p=128))
            sc1_sb = const.tile([128, MT1], F32)
            nc.sync.dma_start(sc1_sb, sc1_d.ap().rearrange("(t p) -> p t", p=128))
            bi1_sb = const.tile([128, MT1], F32)
            nc.sync.dma_start(bi1_sb, bi1_d.ap().rearrange("(t p) -> p t", p=128))

            def load_cv2_consts():
                w2_sb = const.tile([128, KT2, C2], FP8)
                nc.sync.dma_start(
                    w2_sb, w2t_d.ap().rearrange("(kt p) m -> p kt m", p=128)
                )
                sc2_sb = const.tile([128, MT2], F32)
                nc.sync.dma_start(sc2_sb, sc2_d.ap().rearrange("(t p) -> p t", p=128))
                bi2_sb = const.tile([128, MT2], F32)
                nc.sync.dma_start(bi2_sb, bi2_d.ap().rearrange("(t p) -> p t", p=128))
                cng_sb = const.tile([128, 16], F32)
                nc.sync.dma_start(cng_sb, cng_d.ap().rearrange("(t p) -> p t", p=128))
                return w2_sb, sc2_sb, bi2_sb, cng_sb

            # PE HAM warm-up: keep the PE activity window busy from the
            # moment the (tiny, early-landing) sc1 constants arrive until the
            # first real matmul, so the clock gate is already at 8/8 when it
            # issues (the cold ramp shows as 333ns gaps in the trace).
            wps = ps1p.tile([128, 512], F32, tag="ps1")
            for _i in range(80):
                nc.tensor.matmul(
                    wps[0:4, 0:4],
                    sc1_sb,
                    sc1_sb,
                    start=True,
                    stop=True,
                )
            for _i in range(30):
                nc.tensor.matmul(
                    wps[:, 0:32],
                    w1_sb[:, 0, 0:128],
                    w1_sb[:, 0, 0:32],
                    start=True,
                    stop=True,
                )

            pimg = {}  # b -> [P0 list, V8]

            def emit_cv1(b):
                """cv1 + fused BN/SiLU; writes h into padded P0 buffers."""
                P0 = []
                for ct in range(MT1):
                    p0 = pbuf0.tile([128, 40, 44], BF16, tag="P0")
                    nc.gpsimd.memset(p0[:, :, 0:2], NEG)
                    nc.gpsimd.memset(p0[:, :, 42:44], NEG)
                    P0.append(p0)
                v8 = v8p.tile([128, KT2, 40, 40], FP8, tag="V8")
                pimg[b] = [P0, v8]
                for q in range(NQ):
                    xs = xin.tile([128, KT1, QW], BF16, tag="x")
                    nc.sync.dma_start(xs, xv[b][:, :, q * QW:(q + 1) * QW])
                    for mt in range(MT1):
                        ps = ps1p.tile([128, 512], F32, tag="ps1")
                        for kt in range(KT1):
                            nc.tensor.matmul(
                                ps[:, :QW],
                                w1_sb[:, kt, mt * 128:(mt + 1) * 128],
                                xs[:, kt, :],
                                start=(kt == 0),
                                stop=(kt == KT1 - 1),
                            )
                        nc.scalar.activation(
                            out=P0[mt][:, q * 10:(q + 1) * 10, 2:42],
                            in_=ps[:, :QW],
                            func=silu,
                            bias=bi1_sb[:, mt:mt + 1],
                            scale=sc1_sb[:, mt:mt + 1],
                        )

            def emit_pools(b):
                P0, v8 = pimg[b]
                for ct in range(MT1):
                    # fp8 re-encode of h (centered) for cv2's block 0
                    nc.scalar.activation(
                        out=v8[:, ct], in_=P0[ct][:, :, 2:42], func=ident,
                        bias=cng_sb[:, ct:ct + 1],
                    )
                    HX = work.tile([128, 44, 40], BF16, tag="HX")
                    M2 = work.tile([128, 44, 44], BF16, tag="M2")
                    nc.gpsimd.memset(HX[:, 0:2, :], NEG)
                    nc.gpsimd.memset(HX[:, 42:44, :], NEG)
                    p1 = pbuf.tile([128, 40, 44], BF16, tag="P1")
                    p2 = pbuf.tile([128, 40, 44], BF16, tag="P2")
                    p3 = pbuf.tile([128, 40, 40], BF16, tag="P3")
                    for pp in (p1, p2):
                        nc.gpsimd.memset(pp[:, :, 0:2], NEG)
                        nc.gpsimd.memset(pp[:, :, 42:44], NEG)
                    _pools_chain(nc, P0[ct], HX, M2, p1, True)
                    _pools_chain(nc, p1, HX, M2, p2, True)
                    _pools_chain(nc, p2, HX, M2, p3, False)
                    # centered fp8 re-encode: y1 on ACT; delta blocks fused
                    # on DVE as (yk + (-c)) - y_{k-1} -> fp8
                    nc.scalar.activation(
                        out=v8[:, MT1 + ct], in_=p1[:, :, 2:42], func=ident,
                        bias=cng_sb[:, MT1 + ct:MT1 + ct + 1],
                    )
                    nc.vector.scalar_tensor_tensor(
                        out=v8[:, 2 * MT1 + ct], in0=p2[:, :, 2:42],
                        scalar=cng_sb[:, 2 * MT1 + ct:2 * MT1 + ct + 1],
                        in1=p1[:, :, 2:42],
                        op0=mybir.AluOpType.add, op1=mybir.AluOpType.subtract,
                    )
                    nc.vector.scalar_tensor_tensor(
                        out=v8[:, 3 * MT1 + ct], in0=p3,
                        scalar=cng_sb[:, 3 * MT1 + ct:3 * MT1 + ct + 1],
                        in1=p2[:, :, 2:42],
                        op0=mybir.AluOpType.add, op1=mybir.AluOpType.subtract,
                    )

            def emit_cv2(b):
                v8 = pimg[b][1]
                for mt2 in range(MT2):
                    psA = ps2p.tile([128, 2, 512], F32, tag="ps2")
                    psB = ps2p.tile([128, 2, 512], F32, tag="ps2")
                    for kp in range(KP2):
                        lhs = w2_sb[:, 2 * kp:2 * kp + 2, mt2 * 128:(mt2 + 1) * 128]
                        st = kp == 0
                        sp = kp == KP2 - 1
                        for nt, pso in ((0, psA[:, 0, :QW]), (1, psA[:, 1, :QW]),
                                        (2, psB[:, 0, :QW]), (3, psB[:, 1, :QW])):
                            nc.tensor.matmul(
                                pso, lhs,
                                v8[:, 2 * kp:2 * kp + 2, nt * 10:(nt + 1) * 10, :],
                                start=st, stop=sp, perf_mode=DR,
                            )
                    for nt, psrc in ((0, psA[:, 0, :QW]), (1, psA[:, 1, :QW]),
                                     (2, psB[:, 0, :QW]), (3, psB[:, 1, :QW])):
                        oo = osb.tile([128, QW], F32, tag="o")
                        nc.scalar.activation(
                            out=oo, in_=psrc, func=silu,
                            bias=bi2_sb[:, mt2:mt2 + 1],
                            scale=sc2_sb[:, mt2:mt2 + 1],
                        )
                        nc.sync.dma_start(
                            ov[b][:, mt2, nt * QW:(nt + 1) * QW], oo)

            # Software pipeline: cv2(b) is emitted `lag` images behind cv1(b)
            # so the PE always has cv1 work while an image's pool chain +
            # fp8 re-encode completes on DVE/ACT.
            lag = 2 if bl > 2 else 1
            w2_refs = None
            for b in range(bl):
                emit_cv1(b)
                if b == 0:
                    w2_refs = load_cv2_consts()
                    w2_sb, sc2_sb, bi2_sb, cng_sb = w2_refs
                if b >= lag:
                    emit_cv2(b - lag)
                emit_pools(b)
            for b in range(max(0, bl - lag), bl):
                emit_cv2(b)

    nc.compile()
    return nc


_NC_CACHE = {}


def _get_nc(bl=BL):
    if bl not in _NC_CACHE:
        _NC_CACHE[bl] = _build_nc(bl)
    return _NC_CACHE[bl]


def _maxpool5_np(x):
    """x: [C, H, W] f32 -> 5x5 stride-1 pad-2 maxpool."""
    C, HH, WW = x.shape
    xp = np.full((C, HH + 4, WW + 4), -np.inf, np.float32)
    xp[:, 2:-2, 2:-2] = x
    out = np.full((C, HH, WW), -np.inf, np.float32)
    for dy in range(5):
        for dx in range(5):
            np.maximum(out, xp[:, dy:dy + HH, dx:dx + WW], out=out)
    return out


def _prep(inputs):
    """Host-side: quantize weights to ternary, fold BitNet scale + BN into
    per-channel (scale, bias), build the delta-block cv2 weights (fp8) and
    the per-channel centering constants + bias correction."""
    x = np.asarray(inputs["x"], dtype=np.float32)
    w1 = np.asarray(inputs["w1"], dtype=np.float32)
    w2 = np.asarray(inputs["w2"], dtype=np.float32)
    g1 = np.asarray(inputs["g1"], dtype=np.float32)
    b1 = np.asarray(inputs["b1"], dtype=np.float32)
    m1 = np.asarray(inputs["m1"], dtype=np.float32)
    v1 = np.asarray(inputs["v1"], dtype=np.float32)
    g2 = np.asarray(inputs["g2"], dtype=np.float32)
    b2 = np.asarray(inputs["b2"], dtype=np.float32)
    m2 = np.asarray(inputs["m2"], dtype=np.float32)
    v2 = np.asarray(inputs["v2"], dtype=np.float32)

    def fold(w, g, b, m, v):
        s = np.float32(max(np.median(np.abs(w)), EPS))
        t = np.clip(np.round(w / s), -1.0, 1.0).astype(np.float32)
        inv = g / np.sqrt(v + BN_EPS)
        scale = (s * inv).astype(np.float32)
        bias = (b - m * inv).astype(np.float32)
        return t, scale, bias

    t1, sc1, bi1 = fold(w1, g1, b1, m1, v1)
    t2, sc2, bi2 = fold(w2, g2, b2, m2, v2)

    # cv2 delta-block weights: [Wa | Wb+Wc+Wd | Wc+Wd | Wd]
    Wa, Wb, Wc, Wd = (t2[:, i * HID:(i + 1) * HID] for i in range(4))
    Wbcd = Wb + Wc + Wd
    Wcd = Wc + Wd
    w2p = np.concatenate([Wa, Wbcd, Wcd, Wd], axis=1)  # [C2, 4*HID]

    # Calibration: per-channel means of h, y1, y2-y1, y3-y2 from 2 images.
    nb = B if x.shape[0] >= B else x.shape[0]
    cal_imgs = [0, nb // 2] if nb > 1 else [0]
    chs, cy1s, cv2s, cv3s = [], [], [], []
    for bi_ in cal_imgs:
        xb = x[bi_].reshape(C1, S).astype(NPBF16).astype(np.float32)
        ps1 = t1 @ xb
        pre = sc1[:, None] * ps1 + bi1[:, None]
        h = (pre / (1.0 + np.exp(-pre))).astype(NPBF16).astype(np.float32)
        y1 = _maxpool5_np(h.reshape(HID, H, W))
        y2 = _maxpool5_np(y1)
        y3 = _maxpool5_np(y2)
        chs.append(h.mean(axis=1))
        cy1s.append(y1.reshape(HID, S).mean(axis=1))
        cv2s.append((y2 - y1).reshape(HID, S).mean(axis=1))
        cv3s.append((y3 - y2).reshape(HID, S).mean(axis=1))
    ch = np.mean(chs, axis=0).astype(np.float32)
    cy1 = np.mean(cy1s, axis=0).astype(np.float32)
    cv2_ = np.mean(cv2s, axis=0).astype(np.float32)
    cv3_ = np.mean(cv3s, axis=0).astype(np.float32)

    # bias correction: cv2 sees centered blocks, so add back W'@c
    corr = Wa @ ch + Wbcd @ cy1 + Wcd @ cv2_ + Wd @ cv3_
    bi2e = (bi2 + sc2 * corr).astype(np.float32)

    # negated centering constants, packed per V8 k-subtile [16*128]
    cneg = np.concatenate([-ch, -cy1, -cv2_, -cv3_]).astype(np.float32)

    w1t = np.ascontiguousarray(t1.T).astype(NPBF16)
    w2t = np.ascontiguousarray(w2p.T).astype(NPFP8)

    xq = x.reshape(x.shape[0], C1, S).astype(NPBF16)
    shared = dict(w1t=w1t, w2t=w2t, sc1=sc1, bi1=bi1, sc2=sc2, bi2=bi2e,
                  cng=cneg)
    in_maps = []
    for d in range(N_CORES):
        m = dict(shared)
        m["xq"] = np.ascontiguousarray(xq[d * BL:(d + 1) * BL])
        in_maps.append(m)
    return in_maps


def _install_ntff_hook():
    """The agent image's antenv lacks axon_hooks; synthesize it so
    run_bass_kernel_spmd(trace=True) can capture NTFF profiles via the
    axon .so's C ABI (same mechanism trn_boot would install)."""
    import types

    try:
        import antenv.axon_hooks  # noqa: F401

        return
    except ImportError:
        pass
    try:
        import antenv

        bootdir = "/root/.axon_site/trn_agent_boot"
        if bootdir not in sys.path and os.path.isdir(bootdir):
            sys.path.insert(0, bootdir)
        import trn_boot

        hook = trn_boot._ntff_profile_via_ctypes("/opt/axon/libaxon_pjrt.so")
        mod = types.ModuleType("antenv.axon_hooks")
        state = {"h": hook}
        mod.get_axon_ntff_profile_hook = lambda: state["h"]
        mod.set_axon_ntff_profile_hook = lambda h: state.update(h=h)
        sys.modules["antenv.axon_hooks"] = mod
        antenv.axon_hooks = mod
    except Exception as e:  # profiling is best-effort; execution still works
        print(f"ntff hook install failed: {e}", file=sys.stderr)


def _run(inputs, trace=False):
    from concourse import bass_utils

    if trace:
        _install_ntff_hook()
    nc = _get_nc()
    in_maps = _prep(inputs)
    import time

    res = None
    for attempt, delay in ((0, 5), (1, 20), (2, 0)):
        try:
            res = bass_utils.run_bass_kernel_spmd(
                nc, in_maps, core_ids=list(range(N_CORES)), trace=trace,
            )
            break
        except Exception as e:  # transient device errors happen; back off
            if attempt == 2:
                raise
            print(
                f"run_bass_kernel_spmd failed ({type(e).__name__}); "
                f"retrying in {delay}s",
                file=sys.stderr,
            )
            time.sleep(delay)
    assert res is not None
    outs = [res.results[d]["out"] for d in range(N_CORES)]
    full = np.concatenate(outs, axis=0).reshape(B, C2, H, W).astype(np.float32)
    return full, res


def kernel(**inputs):
    full, _ = _run(inputs, trace=False)
    return full


def run_traced(**inputs):
    full, res = _run(inputs, trace=True)
    return full, res.exec_time_ns


# revision 7
# speedup vs baseline: 1.1596x; 1.1272x over previous
"""BitSPPF kernel for Trainium2 (8 NeuronCores, data-parallel over batch).

Pipeline per core (4 images):
  cv1 (1x1 ternary-quantized conv, bf16) -> BN+SiLU (ACT engine)
  -> 3x chained 5x5 maxpool (separable max trees on DVE, bf16)
  -> fp8 re-encode of the SPPF concat -> cv2 in fp8 DoubleRow (2x PE
  throughput) -> BN+SiLU -> DRAM.

cv2's fp8 precision is recovered by two tricks, both free at runtime:
 1. Delta blocks: cat [h,y1,y2,y3] @ [Wa|Wb|Wc|Wd]^T is rewritten as
    [h, y1, y2-y1, y3-y2] @ [Wa|Wb+Wc+Wd|Wc+Wd|Wd]^T (exact identity for
    ternary weights; summed weights stay integers in [-3,3], exact in fp8).
    The delta blocks have small magnitude -> small absolute fp8 error.
 2. Per-channel mean centering: each block is stored as (v - c) in fp8
    with c folded back via the host-computed bias correction
    W'@c added to cv2's BN bias. The subtraction itself rides for free
    on the ACT engine's Identity(scale*x + bias) conversion op.
Calibration constants c come from a 2-image host-side pass in _prep.
"""

import os
import sys

for _p in ("/opt/trn_rl_repo",):
    if _p not in sys.path and os.path.isdir(_p):
        sys.path.insert(0, _p)

import numpy as np
import ml_dtypes

import concourse.bass as bass
import concourse.tile as tile
from concourse import bacc, mybir

BF16 = mybir.dt.bfloat16
F32 = mybir.dt.float32
FP8 = mybir.dt.float8e4
NPBF16 = ml_dtypes.bfloat16
NPFP8 = ml_dtypes.float8_e4m3

# Problem shapes (hardcoded per spec)
B, C1, H, W = 32, 1024, 40, 40
HID, C2 = 512, 1024
S = H * W  # 1600
N_CORES = 8
BL = B // N_CORES  # images per core

NEG = -3.0e38  # effectively -inf for maxpool padding, finite in bf16

EPS = 1e-8
BN_EPS = 1e-5

DR = mybir.MatmulPerfMode.DoubleRow


def _pools_chain(nc, P, HX, M2, Pout, padded_out):
    """One 5x5 stride-1 pad-2 maxpool: P -> Pout.

    P: [128, 40, 44] bf16, data in cols 2..41, cols {0,1,42,43} = NEG.
    HX: [128, 44, 40] scratch; rows {0,1,42,43} pre-set to NEG.
    M2: [128, 44, 44] scratch.
    Pout: [128, 40, 44] (padded_out=True, data to cols 2..41)
          or [128, 40, 40] (padded_out=False).
    """
    nc.vector.tensor_max(M2[:, 0:40, 0:43], P[:, :, 0:43], P[:, :, 1:44])
    nc.vector.tensor_max(HX[:, 2:42, :], M2[:, 0:40, 0:40], M2[:, 0:40, 2:42])
    nc.vector.tensor_max(HX[:, 2:42, :], HX[:, 2:42, :], P[:, :, 4:44])
    nc.vector.tensor_max(M2[:, 0:43, 0:40], HX[:, 0:43, :], HX[:, 1:44, :])
    if padded_out:
        ov = Pout[:, :, 2:42]
    else:
        ov = Pout[:, :, :]
    nc.vector.tensor_max(ov, M2[:, 0:40, 0:40], M2[:, 2:42, 0:40])
    nc.vector.tensor_max(ov, ov, HX[:, 4:44, :])


def _build_nc(bl=BL):
    nc = bacc.Bacc(trn_type="TRN2", debug=False)

    xq_d = nc.dram_tensor("xq", [bl, C1, S], BF16, kind="ExternalInput")
    w1t_d = nc.dram_tensor("w1t", [C1, HID], BF16, kind="ExternalInput")
    w2t_d = nc.dram_tensor("w2t", [4 * HID, C2], FP8, kind="ExternalInput")
    sc1_d = nc.dram_tensor("sc1", [HID], F32, kind="ExternalInput")
    bi1_d = nc.dram_tensor("bi1", [HID], F32, kind="ExternalInput")
    sc2_d = nc.dram_tensor("sc2", [C2], F32, kind="ExternalInput")
    bi2_d = nc.dram_tensor("bi2", [C2], F32, kind="ExternalInput")
    cng_d = nc.dram_tensor("cng", [16 * 128], F32, kind="ExternalInput")
    out_d = nc.dram_tensor("out", [bl, C2, S], F32, kind="ExternalOutput")

    KT1 = C1 // 128       # 8 k-tiles for cv1
    MT1 = HID // 128      # 4 m-tiles (= pool channel tiles)
    KT2 = 4 * HID // 128  # 16 k-subtiles for cv2
    KP2 = KT2 // 2        # 8 fp8 DoubleRow pairs
    MT2 = C2 // 128       # 8 m-tiles for cv2
    NQ = 4                # spatial quarters of 400 cols (10 rows of 40)
    QW = S // NQ          # 400

    xv = xq_d.ap().rearrange("b (kt p) s -> b p kt s", p=128)
    ov = out_d.ap().rearrange("b (mt p) s -> b p mt s", p=128)

    # CoreSim doesn't implement Silu; allow substituting Sigmoid for
    # wiring-validation sim runs (numerics then differ by design).
    if os.environ.get("BITSPPF_SIM_ACT") == "sigmoid":
        silu = mybir.ActivationFunctionType.Sigmoid
    else:
        silu = mybir.ActivationFunctionType.Silu
    ident = mybir.ActivationFunctionType.Identity

    with tile.TileContext(nc) as tc:
        with (
            tc.tile_pool(name="const", bufs=1) as const,
            tc.tile_pool(name="xin", bufs=2) as xin,
            tc.tile_pool(name="pbuf0", bufs=2 * MT1 - 1) as pbuf0,
            tc.tile_pool(name="pbuf", bufs=6) as pbuf,
            tc.tile_pool(name="v8p", bufs=3) as v8p,
            tc.tile_pool(name="work", bufs=1) as work,
            tc.tile_pool(name="osb", bufs=2) as osb,
            tc.tile_pool(name="ps1", bufs=2, space="PSUM") as ps1p,
            tc.tile_pool(name="ps2", bufs=3, space="PSUM") as ps2p,
        ):
            # Pre-warm the ACT engine's Silu spline tables (~2.7us load)
            # during the initial DMA window instead of at the first real
            # activation.
            warm = const.tile([128, 2], F32)
            nc.vector.memset(warm, 0.0)
            nc.scalar.activation(out=warm, in_=warm, func=silu)

            # Load only what cv1(0) needs before its matmuls; the w2
            # load would otherwise delay the first matmul.
            w1_sb = const.tile([128, KT1, HID], BF16)
            nc.sync.dma_start(w1_sb, w1t_d.ap().rearrange("(kt p) m -> p kt m", p=128))
            sc1_sb = const.tile([128, MT1], F32)
            nc.sync.dma_start(sc1_sb, sc1_d.ap().rearrange("(t p) -> p t", p=128))
            bi1_sb = const.tile([128, MT1], F32)
            nc.sync.dma_start(bi1_sb, bi1_d.ap().rearrange("(t p) -> p t", p=128))

            def load_cv2_consts():
                w2_sb = const.tile([128, KT2, C2], FP8)
                nc.sync.dma_start(
                    w2_sb, w2t_d.ap().rearrange("(kt p) m -> p kt m", p=128)
                )
                sc2_sb = const.tile([128, MT2], F32)
                nc.sync.dma_start(sc2_sb, sc2_d.ap().rearrange("(t p) -> p t", p=128))
                bi2_sb = const.tile([128, MT2], F32)
                nc.sync.dma_start(bi2_sb, bi2_d.ap().rearrange("(t p) -> p t", p=128))
                cng_sb = const.tile([128, 16], F32)
                nc.sync.dma_start(cng_sb, cng_d.ap().rearrange("(t p) -> p t", p=128))
                return w2_sb, sc2_sb, bi2_sb, cng_sb

            # PE HAM warm-up: keep the PE activity window busy from the
            # moment the (tiny, early-landing) sc1 constants arrive until the
            # first real matmul, so the clock gate is already at 8/8 when it
            # issues (the cold ramp shows as 333ns gaps in the trace).
            wps = ps1p.tile([128, 512], F32, tag="ps1")
            for _i in range(80):
                nc.tensor.matmul(
                    wps[0:4, 0:4],
                    sc1_sb,
                    sc1_sb,
                    start=True,
                    stop=True,
                )
            for _i in range(30):
                nc.tensor.matmul(
                    wps[:, 0:32],
                    w1_sb[:, 0, 0:128],
                    w1_sb[:, 0, 0:32],
                    start=True,
                    stop=True,
                )

            pimg = {}  # b -> [P0 list, V8]

            def emit_cv1(b):
                """cv1 + fused BN/SiLU; writes h into padded P0 buffers."""
                P0 = []
                for ct in range(MT1):
                    p0 = pbuf0.tile([128, 40, 44], BF16, tag="P0")
                    nc.gpsimd.memset(p0[:, :, 0:2], NEG)
                    nc.gpsimd.memset(p0[:, :, 42:44], NEG)
                    P0.append(p0)
                v8 = v8p.tile([128, KT2, 40, 40], FP8, tag="V8")
                pimg[b] = [P0, v8]
                for q in range(NQ):
                    xs = xin.tile([128, KT1, QW], BF16, tag="x")
                    nc.sync.dma_start(xs, xv[b][:, :, q * QW:(q + 1) * QW])
                    for mt in range(MT1):
                        ps = ps1p.tile([128, 512], F32, tag="ps1")
                        for kt in range(KT1):
                            nc.tensor.matmul(
                                ps[:, :QW],
                                w1_sb[:, kt, mt * 128:(mt + 1) * 128],
                                xs[:, kt, :],
                                start=(kt == 0),
                                stop=(kt == KT1 - 1),
                            )
                        nc.scalar.activation(
                            out=P0[mt][:, q * 10:(q + 1) * 10, 2:42],
                            in_=ps[:, :QW],
                            func=silu,
                            bias=bi1_sb[:, mt:mt + 1],
                            scale=sc1_sb[:, mt:mt + 1],
                        )

            def emit_pools(b):
                P0, v8 = pimg[b]
                for ct in range(MT1):
                    # fp8 re-encode of h (centered) for cv2's block 0
                    nc.scalar.activation(
                        out=v8[:, ct], in_=P0[ct][:, :, 2:42], func=ident,
                        bias=cng_sb[:, ct:ct + 1],
                    )
                    HX = work.tile([128, 44, 40], BF16, tag="HX")
                    M2 = work.tile([128, 44, 44], BF16, tag="M2")
                    nc.gpsimd.memset(HX[:, 0:2, :], NEG)
                    nc.gpsimd.memset(HX[:, 42:44, :], NEG)
                    p1 = pbuf.tile([128, 40, 44], BF16, tag="P1")
                    p2 = pbuf.tile([128, 40, 44], BF16, tag="P2")
                    p3 = pbuf.tile([128, 40, 40], BF16, tag="P3")
                    for pp in (p1, p2):
                        nc.gpsimd.memset(pp[:, :, 0:2], NEG)
                        nc.gpsimd.memset(pp[:, :, 42:44], NEG)
                    _pools_chain(nc, P0[ct], HX, M2, p1, True)
                    _pools_chain(nc, p1, HX, M2, p2, True)
                    _pools_chain(nc, p2, HX, M2, p3, False)
                    # centered fp8 re-encode of y1/y2/y3 on the ACT engine
                    for lvl, src in ((1, p1[:, :, 2:42]), (2, p2[:, :, 2:42]),
                                     (3, p3[:, :, :])):
                        j = lvl * MT1 + ct
                        nc.scalar.activation(
                            out=v8[:, j], in_=src, func=ident,
                            bias=cng_sb[:, j:j + 1],
                        )

            def emit_cv2(b):
                v8 = pimg[b][1]
                for mt2 in range(MT2):
                    psA = ps2p.tile([128, 2, 512], F32, tag="ps2")
                    psB = ps2p.tile([128, 2, 512], F32, tag="ps2")
                    for kp in range(KP2):
                        lhs = w2_sb[:, 2 * kp:2 * kp + 2, mt2 * 128:(mt2 + 1) * 128]
                        st = kp == 0
                        sp = kp == KP2 - 1
                        for nt, pso in ((0, psA[:, 0, :QW]), (1, psA[:, 1, :QW]),
                                        (2, psB[:, 0, :QW]), (3, psB[:, 1, :QW])):
                            nc.tensor.matmul(
                                pso, lhs,
                                v8[:, 2 * kp:2 * kp + 2, nt * 10:(nt + 1) * 10, :],
                                start=st, stop=sp, perf_mode=DR,
                            )
                    for nt, psrc in ((0, psA[:, 0, :QW]), (1, psA[:, 1, :QW]),
                                     (2, psB[:, 0, :QW]), (3, psB[:, 1, :QW])):
                        oo = osb.tile([128, QW], F32, tag="o")
                        nc.scalar.activation(
                            out=oo, in_=psrc, func=silu,
                            bias=bi2_sb[:, mt2:mt2 + 1],
                            scale=sc2_sb[:, mt2:mt2 + 1],
                        )
                        nc.sync.dma_start(
                            ov[b][:, mt2, nt * QW:(nt + 1) * QW], oo)

            # Software pipeline: cv2(b) is emitted `lag` images behind cv1(b)
            # so the PE always has cv1 work while an image's pool chain +
            # fp8 re-encode completes on DVE/ACT.
            lag = 2 if bl > 2 else 1
            w2_refs = None
            for b in range(bl):
                emit_cv1(b)
                if b == 0:
                    w2_refs = load_cv2_consts()
                    w2_sb, sc2_sb, bi2_sb, cng_sb = w2_refs
                if b >= lag:
                    emit_cv2(b - lag)
                emit_pools(b)
            for b in range(max(0, bl - lag), bl):
                emit_cv2(b)

    nc.compile()
    return nc


_NC_CACHE = {}


def _get_nc(bl=BL):
    if bl not in _NC_CACHE:
        _NC_CACHE[bl] = _build_nc(bl)
    return _NC_CACHE[bl]


def _maxpool5_np(x):
    """x: [C, H, W] f32 -> 5x5 stride-1 pad-2 maxpool."""
    C, HH, WW = x.shape
    xp = np.full((C, HH + 4, WW + 4), -np.inf, np.float32)
    xp[:, 2:-2, 2:-2] = x
    out = np.full((C, HH, WW), -np.inf, np.float32)
    for dy in range(5):
        for dx in range(5):
            np.maximum(out, xp[:, dy:dy + HH, dx:dx + WW], out=out)
    return out


def _prep(inputs):
    """Host-side: quantize weights to ternary, fold BitNet scale + BN into
    per-channel (scale, bias), build the delta-block cv2 weights (fp8) and
    the per-channel centering constants + bias correction."""
    x = np.asarray(inputs["x"], dtype=np.float32)
    w1 = np.asarray(inputs["w1"], dtype=np.float32)
    w2 = np.asarray(inputs["w2"], dtype=np.float32)
    g1 = np.asarray(inputs["g1"], dtype=np.float32)
    b1 = np.asarray(inputs["b1"], dtype=np.float32)
    m1 = np.asarray(inputs["m1"], dtype=np.float32)
    v1 = np.asarray(inputs["v1"], dtype=np.float32)
    g2 = np.asarray(inputs["g2"], dtype=np.float32)
    b2 = np.asarray(inputs["b2"], dtype=np.float32)
    m2 = np.asarray(inputs["m2"], dtype=np.float32)
    v2 = np.asarray(inputs["v2"], dtype=np.float32)

    def fold(w, g, b, m, v):
        s = np.float32(max(np.median(np.abs(w)), EPS))
        t = np.clip(np.round(w / s), -1.0, 1.0).astype(np.float32)
        inv = g / np.sqrt(v + BN_EPS)
        scale = (s * inv).astype(np.float32)
        bias = (b - m * inv).astype(np.float32)
        return t, scale, bias

    t1, sc1, bi1 = fold(w1, g1, b1, m1, v1)
    t2, sc2, bi2 = fold(w2, g2, b2, m2, v2)

    # cv2 blocks stay [Wa | Wb | Wc | Wd]; inputs are the centered
    # [h, y1, y2, y3] in fp8
    Wa, Wb, Wc, Wd = (t2[:, i * HID:(i + 1) * HID] for i in range(4))
    w2p = t2

    # Calibration: per-channel means of h, y1, y2, y3 from 2 images.
    nb = x.shape[0]
    cal_imgs = [0, nb // 2] if nb > 1 else [0]
    cals = []
    for bi_ in cal_imgs:
        xb = x[bi_].reshape(C1, S).astype(NPBF16).astype(np.float32)
        ps1 = t1 @ xb
        pre = sc1[:, None] * ps1 + bi1[:, None]
        h = (pre / (1.0 + np.exp(-pre))).astype(NPBF16).astype(np.float32)
        y1 = _maxpool5_np(h.reshape(HID, H, W))
        y2 = _maxpool5_np(y1)
        y3 = _maxpool5_np(y2)
        cals.append([h.mean(axis=1), y1.reshape(HID, S).mean(axis=1),
                     y2.reshape(HID, S).mean(axis=1),
                     y3.reshape(HID, S).mean(axis=1)])
    ch, cy1, cy2_, cy3_ = (
        np.mean([c[i] for c in cals], axis=0).astype(np.float32)
        for i in range(4)
    )

    # bias correction: cv2 sees centered blocks, so add back W@c
    corr = Wa @ ch + Wb @ cy1 + Wc @ cy2_ + Wd @ cy3_
    bi2e = (bi2 + sc2 * corr).astype(np.float32)

    # negated centering constants, packed per V8 k-subtile [16*128]
    cneg = np.concatenate([-ch, -cy1, -cy2_, -cy3_]).astype(np.float32)

    w1t = np.ascontiguousarray(t1.T).astype(NPBF16)
    w2t = np.ascontiguousarray(w2p.T).astype(NPFP8)

    xq = x.reshape(x.shape[0], C1, S).astype(NPBF16)
    shared = dict(w1t=w1t, w2t=w2t, sc1=sc1, bi1=bi1, sc2=sc2, bi2=bi2e,
                  cng=cneg)
    in_maps = []
    for d in range(N_CORES):
        m = dict(shared)
        m["xq"] = np.ascontiguousarray(xq[d * BL:(d + 1) * BL])
        in_maps.append(m)
    return in_maps


def _install_ntff_hook():
    """The agent image's antenv lacks axon_hooks; synthesize it so
    run_bass_kernel_spmd(trace=True) can capture NTFF profiles via the
    axon .so's C ABI (same mechanism trn_boot would install)."""
    import types

    try:
        import antenv.axon_hooks  # noqa: F401

        return
    except ImportError:
        pass
    try:
        import antenv

        bootdir = "/root/.axon_site/trn_agent_boot"
        if bootdir not in sys.path and os.path.isdir(bootdir):
            sys.path.insert(0, bootdir)
        import trn_boot

        hook = trn_boot._ntff_profile_via_ctypes("/opt/axon/libaxon_pjrt.so")
        mod = types.ModuleType("antenv.axon_hooks")
        state = {"h": hook}
        mod.get_axon_ntff_profile_hook = lambda: state["h"]
        mod.set_axon_ntff_profile_hook = lambda h: state.update(h=h)
        sys.modules["antenv.axon_hooks"] = mod
        antenv.axon_hooks = mod
    except Exception as e:  # profiling is best-effort; execution still works
        print(f"ntff hook install failed: {e}", file=sys.stderr)


def _run(inputs, trace=False):
    from concourse import bass_utils

    if trace:
        _install_ntff_hook()
    nc = _get_nc()
    in_maps = _prep(inputs)
    import time

    res = None
    for attempt, delay in ((0, 5), (1, 20), (2, 0)):
        try:
            res = bass_utils.run_bass_kernel_spmd(
                nc, in_maps, core_ids=list(range(N_CORES)), trace=trace,
            )
            break
        except Exception as e:  # transient device errors happen; back off
            if attempt == 2:
                raise
            print(
                f"run_bass_kernel_spmd failed ({type(e).__name__}); "
                f"retrying in {delay}s",
                file=sys.stderr,
            )
            time.sleep(delay)
    assert res is not None
    outs = [res.results[d]["out"] for d in range(N_CORES)]
    full = np.concatenate(outs, axis=0).reshape(B, C2, H, W).astype(np.float32)
    return full, res


def kernel(**inputs):
    full, _ = _run(inputs, trace=False)
    return full


def run_traced(**inputs):
    full, res = _run(inputs, trace=True)
    return full, res.exec_time_ns
